# revision 42
# baseline (speedup 1.0000x reference)
"""Trainium2 Bass kernel: MoE detection head (nn_MoEDetect).

Strategy: data-parallel over batch. 16 samples / 8 cores = 2 samples per core.
Each core runs the full conv trunk (3x3 conv -> SiLU -> 3x3 conv -> SiLU) for
both branches (cv2: 256->64->64, cv3: 256->256->256) at all 3 detection levels,
then a per-sample expert 1x1 conv (MoE). Expert weights are gathered on the
HOST (module_indices -> per-sample [Cout,Cin] matrices) while slicing per-core
inputs, so no on-device routing is needed.

Conv3x3 is computed as 9 shifted 1x1 convs (matmuls) accumulated in PSUM,
reading from zero-padded SBUF activation buffers [C, H+2, W+2]. All matmuls
run in bf16 (fp32 accumulate in PSUM); epilogues (bias + SiLU) run on the
scalar/ACT engine.
"""

import sys
import numpy as np

try:  # concourse is usually on PYTHONPATH already (axon boot)
    import concourse  # noqa: F401
except ImportError:  # fall back to the in-container repo checkout
    sys.path.insert(0, "/opt/trn_rl_repo")

import ml_dtypes  # noqa: E402
from contextlib import ExitStack  # noqa: E402

import concourse.bass as bass  # noqa: E402
import concourse.tile as tile  # noqa: E402
from concourse import bacc, mybir  # noqa: E402
from concourse.bass_utils import run_bass_kernel_spmd  # noqa: E402

BF16 = mybir.dt.bfloat16
F32 = mybir.dt.float32
AF = mybir.ActivationFunctionType

N_CORES = 8
B = 16
SPC = B // N_CORES            # samples per core
CH = 256                      # trunk input channels
C2 = 64                       # cv2 branch width
C3 = 256                      # cv3 branch width
REG = 64                      # 4*reg_max (cv2 expert out)
NCLS = 80                     # nc (cv3 expert out)
M1 = C3 + C2                  # combined conv1 output channels = 320
LEVELS = [(80, 80), (40, 40), (20, 20)]
ROWS = [6, 12, 20]            # output rows per spatial chunk -> N = 480/480/400


def _chunks(H, R):
    return [(r, min(R, H - r)) for r in range(0, H, R)]


def _build(repeat=1):
    nc = bacc.Bacc(
        "TRN2",
        target_bir_lowering=False,
        debug=False,
        enable_asserts=True,
        num_devices=N_CORES,
    )

    x_d, out_d = [], []
    w1_d, w23_d, w22_d, wm3_d, wm2_d, bias_d = [], [], [], [], [], []
    for l, (H, W) in enumerate(LEVELS):
        x_d.append(
            nc.dram_tensor(f"x{l}", [SPC, CH, H, W], BF16, kind="ExternalInput").ap()
        )
        out_d.append(
            nc.dram_tensor(
                f"out{l}", [SPC, REG + NCLS, H, W], F32, kind="ExternalOutput"
            ).ap()
        )
        w1_d.append(
            nc.dram_tensor(f"w1_{l}", [128, 9, 2, M1], BF16, kind="ExternalInput").ap()
        )
        w23_d.append(
            nc.dram_tensor(f"w2c3_{l}", [128, 9, 2, C3], BF16, kind="ExternalInput").ap()
        )
        # cv2 conv2 weights are K-packed: taps (dy,0)+(dy,1) stacked along K
        # (upper half reads a left-shifted copy of the activation), taps
        # (dy,2) in slots 3..5 with a zero upper half.
        w22_d.append(
            nc.dram_tensor(f"w2c2_{l}", [128, 6, C2], BF16, kind="ExternalInput").ap()
        )
        wm3_d.append(
            nc.dram_tensor(
                f"wm3_{l}", [128, SPC, 2, NCLS], BF16, kind="ExternalInput"
            ).ap()
        )
        wm2_d.append(
            nc.dram_tensor(f"wm2_{l}", [C2, SPC, C2], BF16, kind="ExternalInput").ap()
        )
        bias_d.append(
            nc.dram_tensor(f"bias_{l}", [128, 10], F32, kind="ExternalInput").ap()
        )
    # second K-packing for cv2 conv2: slot 0 = row-pair (0,2)+(1,2)
    # against a row-shifted composite, slot 1 = single (2,2)
    w22r_d = [
        nc.dram_tensor(f"w2c2r_{l}", [128, 2, C2], BF16, kind="ExternalInput").ap()
        for l in range(3)
    ]

    with tile.TileContext(nc) as tc, ExitStack() as ctx:
        wpool = ctx.enter_context(tc.tile_pool(name="w", bufs=1))
        xpool = ctx.enter_context(tc.tile_pool(name="x", bufs=1))
        apool = ctx.enter_context(tc.tile_pool(name="a", bufs=1))
        bpool = ctx.enter_context(tc.tile_pool(name="b", bufs=4))
        opool = ctx.enter_context(tc.tile_pool(name="o", bufs=4))
        ppool = ctx.enter_context(tc.tile_pool(name="p", space="PSUM", bufs=2))

        # ---- persistent weight + activation tiles
        w1_sb, w23_sb, w22_sb, wm3_sb, wm2_sb, bias_sb = {}, {}, {}, {}, {}, {}
        xpad, a3p, a2p = {}, {}, {}
        for l, (H, W) in enumerate(LEVELS):
            Hp, Wp = H + 2, W + 2
            bias_sb[l] = wpool.tile([128, 10], F32, name=f"biass{l}")
            w1_sb[l] = wpool.tile([128, 9, 2, M1], BF16, name=f"w1s{l}")
            w23_sb[l] = wpool.tile([128, 9, 2, C3], BF16, name=f"w23s{l}")
            w22_sb[l] = wpool.tile([128, 6, C2], BF16, name=f"w22s{l}")
            wm3_sb[l] = wpool.tile([128, SPC, 2, NCLS], BF16, name=f"wm3s{l}")
            wm2_sb[l] = wpool.tile([C2, SPC, C2], BF16, name=f"wm2s{l}")
            xpad[l] = xpool.tile([128, 2, Hp, Wp], BF16, name=f"xp{l}")
            a3p[l] = apool.tile([128, 2, Hp, Wp], BF16, name=f"a3p{l}")
            a2p[l] = apool.tile([128, Hp, Wp], BF16, name=f"a2p{l}")
        w22r_sb, a2r = {}, {}
        for l, (H, W) in enumerate(LEVELS):
            w22r_sb[l] = wpool.tile([128, 2, C2], BF16, name=f"w22rs{l}")
            a2r[l] = apool.tile([128, H + 2, W + 2], BF16, name=f"a2r{l}")

        def border_memset(t, Hp, Wp):
            # zero only the halo border; interior is fully overwritten
            if len(t.shape) == 4:
                views = [
                    t[:, :, 0:1, 0:Wp], t[:, :, Hp - 1 : Hp, 0:Wp],
                    t[:, :, 0:Hp, 0:1], t[:, :, 0:Hp, Wp - 1 : Wp],
                ]
            else:
                views = [
                    t[:, 0:1, 0:Wp], t[:, Hp - 1 : Hp, 0:Wp],
                    t[:, 0:Hp, 0:1], t[:, 0:Hp, Wp - 1 : Wp],
                ]
            for v in views:
                nc.any.memset(v, 0.0)

        def emit_wdma(l):
            # weight loads on the SP HWDGE queue (SWDGE showed races,
            # Activation-queue issue contends with the Silu epilogues)
            nc.sync.dma_start(w22r_sb[l][:, :, :], w22r_d[l][:, :, :])
            border_memset(a2r[l], LEVELS[l][0] + 2, LEVELS[l][1] + 2)
            nc.sync.dma_start(bias_sb[l][:, :], bias_d[l][:, :])
            for t in range(9):  # per-tap so the first matmuls start early
                nc.sync.dma_start(w1_sb[l][:, t, :, :], w1_d[l][:, t, :, :])
            nc.sync.dma_start(w23_sb[l][:, :, :, :], w23_d[l][:, :, :, :])
            nc.sync.dma_start(w22_sb[l][:, :, :], w22_d[l][:, :, :])
            nc.sync.dma_start(wm3_sb[l][:, :, :, :], wm3_d[l][:, :, :, :])
            nc.sync.dma_start(wm2_sb[l][:, :, :], wm2_d[l][:, :, :])

        def emit_memsets(l):
            H, W = LEVELS[l]
            border_memset(xpad[l], H + 2, W + 2)
            border_memset(a3p[l], H + 2, W + 2)
            border_memset(a2p[l], H + 2, W + 2)

        def emit_xdma(s, l, skip_first=False):
            # chunked by output rows so conv1 of chunk c only waits on the
            # first few row-chunks (startup latency), and WAR release for
            # the next sample is fine-grained
            H, W = LEVELS[l]
            for kt in range(2):
                for ci, (r0, rr) in enumerate(_chunks(H, ROWS[l])):
                    if skip_first and ci == 0:
                        continue
                    nc.sync.dma_start(
                        xpad[l][:, kt, 1 + r0 : 1 + r0 + rr, 1 : 1 + W],
                        x_d[l][s, 128 * kt : 128 * (kt + 1), r0 : r0 + rr, :],
                    )

        # startup-critical loads first: level-0 sample-0 inputs + conv1
        # weights. The l0 weights ride the Activation HWDGE queue (ACT has
        # no Silu work yet at t=0) so they overlap the x chunks on the SP
        # queue; steady-state loads stay on SP to avoid ACT contention.
        emit_memsets(0)
        # first x chunks ahead of the tap split so chunk 0 can start
        H0, W0 = LEVELS[0]
        for kt in range(2):
            nc.sync.dma_start(
                xpad[0][:, kt, 1 : 1 + ROWS[0], 1 : 1 + W0],
                x_d[0][0, 128 * kt : 128 * (kt + 1), 0 : ROWS[0], :],
            )
        # conv1 taps split across both HWDGE queues to double arrival rate
        for t in range(9):
            eng = nc.scalar if t % 2 == 0 else nc.sync
            eng.dma_start(w1_sb[0][:, t, :, :], w1_d[0][:, t, :, :])
        nc.scalar.dma_start(bias_sb[0][:, :], bias_d[0][:, :])
        emit_xdma(0, 0, skip_first=True)
        nc.sync.dma_start(w23_sb[0][:, :, :, :], w23_d[0][:, :, :, :])
        nc.sync.dma_start(w22_sb[0][:, :, :], w22_d[0][:, :, :])
        nc.sync.dma_start(w22r_sb[0][:, :, :], w22r_d[0][:, :, :])
        border_memset(a2r[0], LEVELS[0][0] + 2, LEVELS[0][1] + 2)
        nc.sync.dma_start(wm3_sb[0][:, :, :, :], wm3_d[0][:, :, :, :])
        nc.sync.dma_start(wm2_sb[0][:, :, :], wm2_d[0][:, :, :])
        for l in (1, 2):
            emit_memsets(l)
            emit_wdma(l)

        # conv1 M-chunks: (psum tag, part count, m0, act dest kind)
        M1_CHUNKS = [("p0", 128, 0), ("p1", 128, 128), ("p2", C2, 256)]

        def emit_A_chunk(s, l, r0, rr):
            """conv1 (both branches fused along M) for output rows r0..r0+rr."""
            H, W = LEVELS[l]
            pa = [
                ppool.tile([p, rr, W], F32, tag=tg, name=f"pa{i}")
                for i, (tg, p, _) in enumerate(M1_CHUNKS)
            ]
            for kt in range(2):
                for t in range(9):
                    dy, dx = t // 3, t % 3
                    rhs = xpad[l][:, kt, r0 + dy : r0 + dy + rr, dx : dx + W]
                    first = kt == 0 and t == 0
                    last = kt == 1 and t == 8
                    for i, (_, p, m0) in enumerate(M1_CHUNKS):
                        nc.tensor.matmul(
                            pa[i][:, :, :],
                            w1_sb[l][:, t, kt, m0 : m0 + p],
                            rhs,
                            start=first,
                            stop=last,
                        )
            # bias + SiLU -> padded activation buffers (bf16)
            for kt in range(2):
                nc.scalar.activation(
                    a3p[l][:, kt, 1 + r0 : 1 + r0 + rr, 1 : 1 + W],
                    pa[kt][:, :, :],
                    AF.Silu,
                    bias=bias_sb[l][:, kt : kt + 1],
                )
            nc.scalar.activation(
                a2p[l][:C2, 1 + r0 : 1 + r0 + rr, 1 : 1 + W],
                pa[2][:, :, :],
                AF.Silu,
                bias=bias_sb[l][:C2, 2:3],
            )
            # left-shifted copy into partitions 64..127 for K-packed conv2
            # (cols 0..W: one past the written interior so every cell the
            # zero-padded single taps read is initialized, not garbage)
            nc.sync.dma_start(
                a2p[l][C2:, 1 + r0 : 1 + r0 + rr, 0 : W + 1],
                a2p[l][:C2, 1 + r0 : 1 + r0 + rr, 1 : 2 + W],
            )
            # row-shift composite: lower = copy of the padded activation,
            # upper[h] = pad[h+1] so taps (0,2)+(1,2) pack along K
            nc.sync.dma_start(
                a2r[l][:C2, 1 + r0 : 1 + r0 + rr, 1 : 1 + W],
                a2p[l][:C2, 1 + r0 : 1 + r0 + rr, 1 : 1 + W],
            )
            nc.sync.dma_start(
                a2r[l][C2:, r0 : r0 + rr, 2 : 2 + W],
                a2p[l][:C2, 1 + r0 : 1 + r0 + rr, 2 : 2 + W],
            )

        def emit_B_chunk(s, l, r0, rr):
            """conv2 + MoE 1x1 + output DMA for rows r0..r0+rr of sample s."""
            H, W = LEVELS[l]
            pb0 = ppool.tile([128, rr, W], F32, tag="p0", name="pb0")
            pb1 = ppool.tile([128, rr, W], F32, tag="p1", name="pb1")
            pb2 = ppool.tile([C2, rr, W], F32, tag="p2", name="pb2")
            pb = [pb0, pb1]
            for kt in range(2):
                for t in range(9):
                    dy, dx = t // 3, t % 3
                    rhs = a3p[l][:, kt, r0 + dy : r0 + dy + rr, dx : dx + W]
                    first = kt == 0 and t == 0
                    last = kt == 1 and t == 8
                    for mi in range(2):
                        nc.tensor.matmul(
                            pb[mi][:, :, :],
                            w23_sb[l][:, t, kt, 128 * mi : 128 * (mi + 1)],
                            rhs,
                            start=first,
                            stop=last,
                        )
            for t in range(3):  # column pairs (dy,0)+(dy,1)
                nc.tensor.matmul(
                    pb2[:, :, :],
                    w22_sb[l][:, t, :],
                    a2p[l][:, r0 + t : r0 + t + rr, 0:W],
                    start=t == 0,
                    stop=False,
                )
            # row pair (0,2)+(1,2) against the row-shift composite
            nc.tensor.matmul(
                pb2[:, :, :],
                w22r_sb[l][:, 0, :],
                a2r[l][:, r0 : r0 + rr, 2 : 2 + W],
                start=False,
                stop=False,
            )
            # remaining single (2,2) at K=64
            nc.tensor.matmul(
                pb2[:, :, :],
                w22r_sb[l][:C2, 1, :],
                a2p[l][:C2, r0 + 2 : r0 + 2 + rr, 2 : 2 + W],
                start=False,
                stop=True,
            )
            b3c = bpool.tile([128, 2, rr, W], BF16, tag="b3", name="b3c")
            b2c = bpool.tile([C2, rr, W], BF16, tag="b2", name="b2c")
            for kt in range(2):
                nc.scalar.activation(
                    b3c[:, kt, :, :],
                    pb[kt][:, :, :],
                    AF.Silu,
                    bias=bias_sb[l][:, 3 + kt : 4 + kt],
                )
            nc.scalar.activation(
                b2c[:, :, :], pb2[:, :, :], AF.Silu, bias=bias_sb[l][:C2, 5:6]
            )
            # MoE 1x1 with this sample's expert weights
            pm3 = ppool.tile([NCLS, rr, W], F32, tag="p3", name="pm3")
            nc.tensor.matmul(
                pm3[:, :, :], wm3_sb[l][:, s, 0, :], b3c[:, 0, :, :],
                start=True, stop=False,
            )
            nc.tensor.matmul(
                pm3[:, :, :], wm3_sb[l][:, s, 1, :], b3c[:, 1, :, :],
                start=False, stop=True,
            )
            pm2 = ppool.tile([C2, rr, W], F32, tag="p3", name="pm2")
            nc.tensor.matmul(
                pm2[:, :, :], wm2_sb[l][:, s, :], b2c[:, :, :], start=True, stop=True
            )
            # bias add on DVE (keeps ACT exclusively on Silu; avoids
            # activation-table swaps on real HW)
            o3 = opool.tile([NCLS, rr, W], F32, tag="o3", name="o3")
            nc.vector.tensor_scalar_add(
                o3[:, :, :], pm3[:, :, :], bias_sb[l][:NCLS, 6 + s : 7 + s]
            )
            o2 = opool.tile([C2, rr, W], F32, tag="o2", name="o2")
            nc.vector.tensor_scalar_add(
                o2[:, :, :], pm2[:, :, :], bias_sb[l][:C2, 8 + s : 9 + s]
            )
            nc.sync.dma_start(out_d[l][s, 0:C2, r0 : r0 + rr, :], o2[:, :, :])
            nc.sync.dma_start(out_d[l][s, C2:, r0 : r0 + rr, :], o3[:, :, :])

        # ---- software-pipelined emission: B(unit i) interleaved with A(unit i+1)
        # (repeat>1 re-runs the whole batch body for device-time measurement)
        units = [(s, l) for s in range(SPC) for l in range(3)] * repeat

        def a_emitters(u):
            s, l = u
            H, _ = LEVELS[l]
            return [
                (lambda s=s, l=l, r0=r0, rr=rr: emit_A_chunk(s, l, r0, rr))
                for r0, rr in _chunks(H, ROWS[l])
            ]

        def b_emitters(u):
            s, l = u
            H, _ = LEVELS[l]
            return [
                (lambda s=s, l=l, r0=r0, rr=rr: emit_B_chunk(s, l, r0, rr))
                for r0, rr in _chunks(H, ROWS[l])
            ]

        # (x-DMA for units[0] already emitted in the startup block)
        for f in a_emitters(units[0]):
            f()
        for i, u in enumerate(units):
            nxt = units[i + 1] if i + 1 < len(units) else None
            blist = b_emitters(u)
            alist = []
            if nxt is not None:
                emit_xdma(*nxt)
                alist = a_emitters(nxt)
            # proportional round-robin merge of blist and alist
            nb, na = len(blist), len(alist)
            bi = ai = 0
            while bi < nb or ai < na:
                if ai >= na or (bi < nb and bi * max(na, 1) <= ai * max(nb, 1)):
                    blist[bi]()
                    bi += 1
                else:
                    alist[ai]()
                    ai += 1

    nc.compile()
    return nc


_CACHE = {}


def _get_nc(repeat=1):
    key = f"nc{repeat}"
    if key not in _CACHE:
        _CACHE[key] = _build(repeat)
    return _CACHE[key]


def _prep_in_maps(inp):
    bf = ml_dtypes.bfloat16
    idx = np.asarray(inp["module_indices"]).astype(np.int64)

    shared = {}
    for l in range(3):
        # conv1: fuse cv3 (256 out) + cv2 (64 out) along M -> lhsT [128,9,2,320]
        w3 = np.asarray(inp["cv3_w1"][l]).transpose(2, 3, 1, 0).reshape(9, CH, C3)
        wc2 = np.asarray(inp["cv2_w1"][l]).transpose(2, 3, 1, 0).reshape(9, CH, C2)
        comb = np.concatenate([w3, wc2], axis=2)          # [tap, cin, 320]
        comb = comb.reshape(9, 2, 128, M1).transpose(2, 0, 1, 3)
        shared[f"w1_{l}"] = np.ascontiguousarray(comb).astype(bf)
        w23 = (
            np.asarray(inp["cv3_w2"][l])
            .transpose(2, 3, 1, 0)
            .reshape(9, C3, C3)
            .reshape(9, 2, 128, C3)
            .transpose(2, 0, 1, 3)
        )
        shared[f"w2c3_{l}"] = np.ascontiguousarray(w23).astype(bf)
        w22 = (
            np.asarray(inp["cv2_w2"][l]).transpose(2, 3, 1, 0).reshape(9, C2, C2)
        )  # [tap, cin, cout]
        w22p = np.zeros((128, 6, C2), np.float32)
        for dy in range(3):
            w22p[:C2, dy] = w22[dy * 3 + 0]
            w22p[C2:, dy] = w22[dy * 3 + 1]
            w22p[:C2, 3 + dy] = w22[dy * 3 + 2]
        shared[f"w2c2_{l}"] = np.ascontiguousarray(w22p).astype(bf)
        w22r = np.zeros((128, 2, C2), np.float32)
        w22r[:C2, 0] = w22[0 * 3 + 2]   # tap (0,2) on the normal lower half
        w22r[C2:, 0] = w22[1 * 3 + 2]   # tap (1,2) on the row-shifted upper
        w22r[:C2, 1] = w22[2 * 3 + 2]   # tap (2,2), used at K=64
        shared[f"w2c2r_{l}"] = np.ascontiguousarray(w22r).astype(bf)

    in_maps = []
    for c in range(N_CORES):
        m = dict(shared)
        sl = slice(SPC * c, SPC * (c + 1))
        e = idx[sl]
        for l in range(3):
            m[f"x{l}"] = np.asarray(inp[f"x{l}"])[sl].astype(bf)
            we3 = np.asarray(inp["cv3_we"][l])[e]          # [SPC, 80, 256]
            t = we3.reshape(SPC, NCLS, 2, 128).transpose(3, 0, 2, 1)
            m[f"wm3_{l}"] = np.ascontiguousarray(t).astype(bf)
            we2 = np.asarray(inp["cv2_we"][l])[e]          # [SPC, 64, 64]
            m[f"wm2_{l}"] = np.ascontiguousarray(we2.transpose(2, 0, 1)).astype(bf)
            bias = np.zeros((128, 10), np.float32)
            b13 = np.asarray(inp["cv3_b1"][l])
            bias[:, 0] = b13[:128]
            bias[:, 1] = b13[128:]
            bias[:C2, 2] = np.asarray(inp["cv2_b1"][l])
            b23 = np.asarray(inp["cv3_b2"][l])
            bias[:, 3] = b23[:128]
            bias[:, 4] = b23[128:]
            bias[:C2, 5] = np.asarray(inp["cv2_b2"][l])
            for s in range(SPC):
                bias[:NCLS, 6 + s] = np.asarray(inp["cv3_be"][l][e[s]])
                bias[:C2, 8 + s] = np.asarray(inp["cv2_be"][l][e[s]])
            m[f"bias_{l}"] = bias
        in_maps.append(m)
    return in_maps


def _run(inp, **kwargs):
    nc = _get_nc()
    in_maps = _prep_in_maps(inp)
    br = run_bass_kernel_spmd(nc, in_maps, list(range(N_CORES)), **kwargs)
    outs = []
    for l in range(3):
        outs.append(
            np.concatenate([br.results[c][f"out{l}"] for c in range(N_CORES)], axis=0)
        )
    return tuple(outs), br


def kernel(**inputs):
    outs, _ = _run(inputs)
    return outs
